# revision 15
# baseline (speedup 1.0000x reference)
"""Trainium2 Bass kernel for nn_AttentionNetwork (B=4096, S=200, D_in=256, D_out=128).

Math (per batch b):
    K = x@Wk + bk ; V = x@Wv + bv
    scores = Q . K        -> softmax over s (invariant to the bk term, dropped)
    atten = softmax(scores) * NORM          (output 2)
    out = sum_s atten[s] * V[s] = (atten @ x) @ Wv + NORM * bv   (output 1)

Design (data-parallel over B across 8 cores, 512 batches/core):
  * scores via qk := Q @ Wk.T :  scores[b,s] = sum_i qk[b,i] * x[b,s,i].
    PE computes this with qk^T as the stationary and x^T (on-chip PE
    transpose) as the moving operand.  To get scores for batch b on PSUM
    partition b (compact block layout for a batched softmax), the stationary
    for each 2-batch group is the full [128i, BLKb] qk^T block with all
    columns zeroed except the group's two -> each matmul contributes exact
    zeros elsewhere and all 2*NG matmuls accumulate into one PSUM bank.
  * c := atten @ x with the same masked-stationary trick using atten^T
    (contraction over s, x used in natural layout - no second transpose).
  * out^T[d,b] = Wv^T @ c^T + NORM*bv, accumulated into a d-major SBUF
    tile; one DMA at the end; the host transposes [DO, nb] -> [nb, DO].
All matmuls run as float32r (the PE rounds operands to ~12 mantissa bits in
both fp32 modes; float32r streams at full rate for free dim >= 256).
"""
import numpy as np
from contextlib import ExitStack

import concourse.bass as bass
import concourse.bacc as bacc
import concourse.mybir as mybir
import concourse.tile as tile
from concourse import bass_utils
import bass_rust

B, S, DI, DO = 4096, 200, 256, 128
NORM = float(1.0 / np.float32(np.sqrt(np.float32(DO))))
NCORES = 8
NB = B // NCORES          # batches per core
S0, S1 = 128, S - 128     # s-chunks: 128 + 72
f32 = mybir.dt.float32
f32r = mybir.dt.float32r

BLK = 32                  # batches per block (softmax batch)
NG = BLK // 2             # 2-batch groups per block
SPREAD_STEP = BLK + 2     # stride between groups' mask positions


def _strided(ap, dims):
    """Copy of `ap` with free dims replaced by `dims` [[step,count],..]."""
    v = ap.copy()
    v.ap = bass_rust.VecI64Pair([list(v.ap[0])] + [list(d) for d in dims])
    return v


def rr(ap):
    return ap.bitcast(f32r)


def build(nc, nb=NB):
    nblk = nb // BLK
    x_d = nc.dram_tensor("x", [nb, S, DI], f32, kind="ExternalInput")
    q_d = nc.dram_tensor("q", [nb, DO], f32, kind="ExternalInput")
    wkt_d = nc.dram_tensor("wkt", [DO, DI], f32, kind="ExternalInput")
    wv_d = nc.dram_tensor("wv", [DI, DO], f32, kind="ExternalInput")
    bvn_d = nc.dram_tensor("bvn", [DO, 1], f32, kind="ExternalInput")
    eye_d = nc.dram_tensor("eye", [128, 128], f32, kind="ExternalInput")
    msk_d = nc.dram_tensor("msk", [128, 1], mybir.dt.int32, kind="ExternalInput")  # 1 on even rows
    out_d = nc.dram_tensor("out", [DO, nb], f32, kind="ExternalOutput")
    att_d = nc.dram_tensor("att", [nb, S], f32, kind="ExternalOutput")

    with tile.TileContext(nc) as tc, ExitStack() as ctx:
        const = ctx.enter_context(tc.tile_pool(name="const", bufs=1))
        xpool = ctx.enter_context(tc.tile_pool(name="xp", bufs=2 * NG))
        sb = ctx.enter_context(tc.tile_pool(name="sb", bufs=3))
        blkb = ctx.enter_context(tc.tile_pool(name="blkb", bufs=2))
        pt = ctx.enter_context(tc.tile_pool(name="pt", bufs=4, space="PSUM"))
        psS = ctx.enter_context(tc.tile_pool(name="psS", bufs=2, space="PSUM"))
        psC = ctx.enter_context(tc.tile_pool(name="psC", bufs=2, space="PSUM"))

        eye = const.tile([128, 128], f32, tag="eye")
        nc.sync.dma_start(rr(eye[:]), rr(eye_d[:]))
        wkt = const.tile([DO, DI], f32, tag="wkt")
        nc.sync.dma_start(rr(wkt[:]), rr(wkt_d[:]))
        wv = const.tile([128, 2, DO], f32, tag="wv")
        nc.sync.dma_start(rr(wv[:]), rr(wv_d.ap().rearrange("(c p) d -> p c d", p=128)))
        bvn = const.tile([DO, 1], f32, tag="bvn")
        nc.sync.dma_start(bvn[:], bvn_d[:])
        msk = const.tile([128, 1], mybir.dt.int32, tag="msk")
        nc.sync.dma_start(msk[:], msk_d[:])
        outall = const.tile([DO, nb], f32, tag="outall")

        # Masked stationaries (double-buffered by block parity); zeroed once —
        # the spread pattern rewrites the same positions every block.
        mq = [const.tile([128, 2, NG * BLK], f32, tag=f"mq{p}", name=f"mq{p}")
              for p in range(2)]
        ma0 = [const.tile([S0, NG * BLK], f32, tag=f"ma0{p}", name=f"ma0{p}")
               for p in range(2)]
        ma1 = [const.tile([S1, NG * BLK], f32, tag=f"ma1{p}", name=f"ma1{p}")
               for p in range(2)]
        zcol = const.tile([128, 1], f32, tag="zcol")
        nc.vector.memset(zcol[:], 0.0)
        for t in mq + ma0 + ma1:
            p, fs = t.shape[0], int(np.prod(t.shape[1:]))
            flat = t[:].rearrange("p ... -> p (...)") if len(t.shape) > 2 else t[:]
            nc.vector.tensor_copy(rr(flat), rr(zcol[0:p, :].broadcast_to((p, fs))))

        for blk in range(nblk):
            b0 = blk * BLK
            par = blk % 2
            # ---- phase A: q -> qk^T -> masked stationaries ----------------
            qb = sb.tile([BLK, DO], f32, tag="qb")
            nc.sync.dma_start(rr(qb[:]), rr(q_d[b0:b0 + BLK, :]))
            qt_ps = pt.tile([DO, BLK], f32, tag="pt")
            nc.tensor.transpose(rr(qt_ps[:]), rr(qb[:]), rr(eye[0:BLK, 0:BLK]))
            qt = sb.tile([DO, BLK], f32, tag="qt")
            nc.vector.tensor_copy(rr(qt[:]), rr(qt_ps[:]))
            for h in range(2):
                qk_ps = pt.tile([128, BLK], f32, tag="pt")
                nc.tensor.matmul(qk_ps[:], rr(wkt[:, h * 128:(h + 1) * 128]),
                                 rr(qt[:]), start=True, stop=True)
                # spread each group's 2 columns into the masked tile
                dest = _strided(rr(mq[par][:, h, :]), [[SPREAD_STEP, NG], [1, 2]])
                src = rr(qk_ps[:].rearrange("p (g d) -> p g d", d=2))
                nc.vector.tensor_copy(dest, src)

            # ---- phase B: per 2-batch group: load x, transpose, scores ----
            sc_ps = psS.tile([BLK, 2 * S], f32, tag="sc")
            xas, xbs = [], []
            for g in range(NG):
                bb = b0 + 2 * g
                xa = xpool.tile([S0, 2, DI], f32, tag="xa")
                nc.sync.dma_start(rr(xa[:]),
                                  rr(x_d.ap()[bb:bb + 2, 0:S0, :].transpose([1, 0, 2])))
                xb = xpool.tile([S1, 2, DI], f32, tag="xb")
                nc.sync.dma_start(rr(xb[:]),
                                  rr(x_d.ap()[bb:bb + 2, S0:S, :].transpose([1, 0, 2])))
                xas.append(xa)
                xbs.append(xb)
                for h in range(2):
                    xt_ps = pt.tile([128, 2, S], f32, tag="pt")
                    hs = slice(h * 128, (h + 1) * 128)
                    for beta in range(2):
                        nc.tensor.transpose(rr(xt_ps[:, beta, 0:S0]),
                                            rr(xa[:, beta, hs]), rr(eye[:]))
                        nc.tensor.transpose(rr(xt_ps[:, beta, S0:S]),
                                            rr(xb[:, beta, hs]), rr(eye[0:S1, 0:S1]))
                    xt = sb.tile([128, 2, S], f32, tag=f"xt{h}")
                    if h == 0:
                        nc.vector.tensor_copy(rr(xt[:]), rr(xt_ps[:]))
                    else:
                        nc.scalar.copy(rr(xt[:]), rr(xt_ps[:]))
                    nc.tensor.matmul(
                        sc_ps[:],
                        rr(mq[par][:, h, g * BLK:(g + 1) * BLK]),
                        rr(xt[:].rearrange("p b s -> p (b s)")),
                        start=(g == 0 and h == 0), stop=(g == NG - 1 and h == 1))

            # ---- phase C: softmax over the block --------------------------
            # compact scores: even batches sit at cols 0:S, odd at S:2S
            # merge the two segments: odd rows valid in seg 1, even in seg 0
            sc = blkb.tile([BLK, S], f32, tag="sc_sb")
            nc.vector.tensor_copy(sc[:], sc_ps[:, S:2 * S])
            nc.vector.copy_predicated(sc[:], msk[0:BLK, :].broadcast_to((BLK, S)),
                                      sc_ps[:, 0:S])
            negmax = sb.tile([BLK, 1], f32, tag="negmax")
            nc.vector.tensor_reduce(negmax[:], sc[:], axis=mybir.AxisListType.X,
                                    op=mybir.AluOpType.max, negate=True)
            ex = blkb.tile([BLK, S], f32, tag="ex")
            z = sb.tile([BLK, 1], f32, tag="z")
            nc.scalar.activation(ex[:], sc[:], mybir.ActivationFunctionType.Exp,
                                 bias=negmax[:], accum_out=z[:])
            rz = sb.tile([BLK, 1], f32, tag="rz")
            nc.vector.reciprocal(rz[:], z[:])
            rzn = sb.tile([BLK, 1], f32, tag="rzn")
            nc.vector.tensor_scalar_mul(rzn[:], rz[:], NORM)
            att = blkb.tile([BLK, S], f32, tag="att")
            nc.vector.tensor_scalar_mul(att[:], ex[:], rzn[:])
            nc.sync.dma_start(att_d[b0:b0 + BLK, :], att[:])
            attr = blkb.tile([BLK, S], f32, tag="attr")   # f32r twin for the PE
            nc.vector.tensor_copy(rr(attr[:]), rr(att[:]))

            # atten^T -> masked stationaries
            at0_ps = pt.tile([S0, BLK], f32, tag="pt")
            nc.tensor.transpose(rr(at0_ps[:]), rr(attr[:, 0:S0]), rr(eye[0:BLK, 0:BLK]))
            dest = _strided(rr(ma0[par][:, :]), [[SPREAD_STEP, NG], [1, 2]])
            nc.vector.tensor_copy(dest, rr(at0_ps[:].rearrange("p (g d) -> p g d", d=2)))
            at1_ps = pt.tile([S1, BLK], f32, tag="pt")
            nc.tensor.transpose(rr(at1_ps[:]), rr(attr[:, S0:S]), rr(eye[0:BLK, 0:BLK]))
            dest = _strided(rr(ma1[par][:, :]), [[SPREAD_STEP, NG], [1, 2]])
            nc.vector.tensor_copy(dest, rr(at1_ps[:].rearrange("p (g d) -> p g d", d=2)))

            # ---- phase D: c = atten @ x (masked accumulation) -------------
            c_ps = psC.tile([BLK, 2 * DI], f32, tag="c")
            for g in range(NG):
                gs = slice(g * BLK, (g + 1) * BLK)
                nc.tensor.matmul(c_ps[:], rr(ma0[par][:, gs]),
                                 rr(xas[g][:].rearrange("p b i -> p (b i)")),
                                 start=(g == 0), stop=False)
                nc.tensor.matmul(c_ps[:], rr(ma1[par][:, gs]),
                                 rr(xbs[g][:].rearrange("p b i -> p (b i)")),
                                 start=False, stop=(g == NG - 1))

            # ---- phase E: out^T = Wv^T @ c^T + NORM*bv --------------------
            csb = blkb.tile([BLK, DI], f32, tag="csb")
            nc.vector.tensor_copy(csb[:], c_ps[:, DI:2 * DI])
            nc.vector.copy_predicated(csb[:], msk[0:BLK, :].broadcast_to((BLK, DI)),
                                      c_ps[:, 0:DI])
            oT_ps = pt.tile([DO, BLK], f32, tag="pt")
            for h in range(2):
                ct_ps = pt.tile([128, BLK], f32, tag="pt")
                nc.tensor.transpose(ct_ps[:], csb[:, h * 128:(h + 1) * 128],
                                    eye[0:BLK, 0:BLK])
                ct = sb.tile([128, BLK], f32, tag=f"ct{h}")
                nc.vector.tensor_copy(rr(ct[:]), rr(ct_ps[:]))
                nc.tensor.matmul(oT_ps[:], rr(wv[:, h, :]), rr(ct[:]),
                                 start=(h == 0), stop=(h == 1))
            nc.vector.tensor_scalar_add(outall[:, b0:b0 + BLK], oT_ps[:], bvn[:])

        nc.sync.dma_start(out_d[:], outall[:])
    return nc


def kernel(x, Q, Wk, bk, Wv, bv, nb=NB, cores=NCORES, trace=False):
    x = np.ascontiguousarray(x, dtype=np.float32)
    q2 = np.ascontiguousarray(Q.reshape(Q.shape[0], DO), dtype=np.float32)
    wkt = np.ascontiguousarray(Wk.T, dtype=np.float32)
    wvc = np.ascontiguousarray(Wv, dtype=np.float32)
    bvn = np.ascontiguousarray(bv.reshape(DO, 1) * np.float32(NORM), dtype=np.float32)
    eye = np.eye(128, dtype=np.float32)
    msk = (np.arange(128) % 2 == 0).astype(np.int32).reshape(128, 1)

    nc = bacc.Bacc("TRN2", target_bir_lowering=False, debug=False)
    build(nc, nb=nb)
    nc.compile()
    in_maps = []
    for c in range(cores):
        lo = c * nb
        in_maps.append({
            "x": x[lo:lo + nb],
            "q": q2[lo:lo + nb],
            "wkt": wkt, "wv": wvc, "bvn": bvn, "eye": eye, "msk": msk,
        })
    res = bass_utils.run_bass_kernel_spmd(nc, in_maps, core_ids=list(range(cores)),
                                          trace=trace)
    out = np.concatenate([r["out"].T for r in res.results], axis=0)
    att = np.concatenate([r["att"] for r in res.results], axis=0)
    if trace:
        kernel.last_results = res
    return out, att.reshape(att.shape[0], 1, S)


if __name__ == "__main__":
    rng = np.random.default_rng(0)
    nb = 64
    x = rng.standard_normal((nb, S, DI), dtype=np.float32)
    Q = rng.standard_normal((nb, 1, DO), dtype=np.float32)
    s = np.float32(1.0 / np.sqrt(DI))
    Wk = rng.standard_normal((DI, DO), dtype=np.float32) * s
    bk = rng.standard_normal((DO,), dtype=np.float32) * s
    Wv = rng.standard_normal((DI, DO), dtype=np.float32) * s
    bv = rng.standard_normal((DO,), dtype=np.float32) * s
    out, att = kernel(x, Q, Wk, bk, Wv, bv, nb=nb, cores=1)
    K = x @ Wk + bk
    V = x @ Wv + bv
    sc = np.einsum("bd,bsd->bs", Q[:, 0, :], K)
    m = sc.max(axis=1, keepdims=True)
    e = np.exp(sc - m)
    a_ref = e / e.sum(axis=1, keepdims=True) * NORM
    o_ref = np.einsum("bs,bsd->bd", a_ref, V)
    print("out relerr:", np.abs(out - o_ref).max() / np.abs(o_ref).max())
    print("att relerr:", np.abs(att[:, 0, :] - a_ref).max() / np.abs(a_ref).max())
